# revision 9
# baseline (speedup 1.0000x reference)
"""Trainium2 Bass kernel for DanceDecoder: 2-layer autoregressive LSTM.

B=8192, T=60, HID=512, OUT=51, LAT=64.  Data-parallel over 8 cores
(1024 batch rows each).  Feature-major layout: features on SBUF
partitions, batch in the free dimension (2 blocks of 512 columns).

v2: recurrent matmuls run in fp8e4m3 with DoubleRow perf mode (2 k-tiles
of 128 per instruction, 0.5 cycles/row), weights pre-scaled by 2^8 and
descaled inside the gate activation's scale parameter.  Gate biases ride
a constant-one fp8 row in the contraction (no per-gate Act bias), which
lets one [128,1536]-wide sigmoid cover i/f/o per cell.  Gate values and
the c state are fp16 so DVE elementwise ops hit the 2x perf mode; h is
staged fp16 then cast to the fp8 state tile on GpSimd.  h1/h2 fp8 state
is double-buffered per step (ping-pong), so no commit copies and no
read-before-write hazards.  fc_out runs in fp16 from the fp16 h2 copy.
"""
import sys
sys.path.insert(0, "/opt/trn_rl_repo")

import numpy as np
import ml_dtypes
import concourse.bacc as bacc
import concourse.mybir as mybir
import concourse.tile as tile
from concourse.bass_utils import run_bass_kernel_spmd

HID = 512
OUT = 51
LAT = 64
T = 60
B = 8192
NCORES = 8
BC = B // NCORES          # 1024 batch columns per core
NBLK = 2                  # column blocks per core
NB = BC // NBLK           # 512 columns per block
KH = HID // 128           # 4 hidden chunks
SW = 256.0                # fp8 weight pre-scale (descaled in activation)

F32 = mybir.dt.float32
F32R = mybir.dt.float32r
F16 = mybir.dt.float16
F8 = mybir.dt.float8e4
AF = mybir.ActivationFunctionType
OP = mybir.AluOpType
DR = mybir.MatmulPerfMode.DoubleRow
E4M3 = ml_dtypes.float8_e4m3

# S-tile (fp8 state) layout along dim1 (20 k-tiles of 128 partitions):
#  0-3  h1 (even steps)    4-7  h1 (odd steps)
#  8    x rows 0-50, ones row 51, 1/16 row 52 (bias residual), zeros 53-127
#  9    zeros (pad partner of tile 8)
# 10-13 h2 (even steps)   14-17 h2 (odd steps)
# 18    ones row 0, 1/16 row 1 (bias residual), zeros elsewhere
# 19    zeros (pad partner of tile 18)
# The bias riding the ones row is fp8; a second row carrying 16x the
# quantization residual against a 1/16 input cuts the systematic bias
# error ~16x (it dominated the end-to-end error).
NS = 20

_cached = {}


def build_module(unroll_T=T):
    nc = bacc.Bacc("TRN2", target_bir_lowering=False, debug=False)

    # ---- DRAM I/O (per core) ----
    zT = nc.dram_tensor("zT", [LAT, BC], F32R, kind="ExternalInput")
    s0 = nc.dram_tensor("s0", [128, NS, BC], F8, kind="ExternalInput")
    w0d = nc.dram_tensor("w0d", [128, 6, 4 * HID], F8, kind="ExternalInput")
    w1d = nc.dram_tensor("w1d", [128, 10, 4 * HID], F8, kind="ExternalInput")
    wod = nc.dram_tensor("wod", [128, KH, OUT], F16, kind="ExternalInput")
    wid = nc.dram_tensor("wid", [LAT, 2 * HID], F32R, kind="ExternalInput")
    bid = nc.dram_tensor("bid", [128, 8], F32, kind="ExternalInput")
    bod = nc.dram_tensor("bod", [OUT, 1], F32, kind="ExternalInput")
    frames = nc.dram_tensor("frames", [unroll_T, OUT, BC], F32,
                            kind="ExternalOutput")

    with tile.TileContext(nc) as tc:
        with (
            tc.tile_pool(name="wpool", bufs=1) as wp,
            tc.tile_pool(name="tmp", bufs=3) as tp,
            tc.tile_pool(name="psum", bufs=2, space="PSUM") as pp,
        ):
            # ---- persistent SBUF tiles ----
            W0 = wp.tile([128, 6, 4 * HID], F8, tag="W0")
            W1 = wp.tile([128, 10, 4 * HID], F8, tag="W1")
            WO = wp.tile([128, KH, OUT], F16, tag="WO")
            WI = wp.tile([LAT, 2 * HID], F32R, tag="WI")
            S = wp.tile([128, NS, BC], F8, tag="S")
            BI = wp.tile([128, 8], F32, tag="BI")
            BO = wp.tile([OUT, 1], F32, tag="BO")
            ZT = wp.tile([LAT, BC], F32R, tag="ZT")
            C1 = wp.tile([128, KH, BC], F16, tag="C1")
            C2 = wp.tile([128, KH, BC], F16, tag="C2")
            H1F = wp.tile([128, KH, BC], F16, tag="H1F")
            H2F = wp.tile([128, KH, BC], F16, tag="H2F")
            XF = wp.tile([OUT, BC], F32, tag="XF")

            # ---- load everything ----
            nc.sync.dma_start(W0[:], w0d[:])
            nc.sync.dma_start(W1[:], w1d[:])
            nc.sync.dma_start(WO[:], wod[:])
            nc.sync.dma_start(WI[:], wid[:])
            nc.sync.dma_start(S[:], s0[:])
            nc.sync.dma_start(BI[:], bid[:])
            nc.sync.dma_start(BO[:], bod[:])
            nc.sync.dma_start(ZT[:], zT[:])

            # ---- init: h0/c0 = fc_init(z), h0 into both layers' fp8 state,
            # c0 into both fp16 c tiles ----
            for b in range(NBLK):
                s = b * NB
                for m in range(8):
                    acc = pp.tile([128, NB], F32, tag="G1")
                    nc.tensor.matmul(acc[:], WI[:, m * 128:(m + 1) * 128],
                                     ZT[:, s:s + NB], start=True, stop=True)
                    if m < KH:
                        ht = tp.tile([128, NB], F16, tag="ht")
                        nc.vector.tensor_scalar(ht[:], acc[:], BI[:, m:m + 1],
                                                None, OP.add)
                        nc.gpsimd.tensor_copy(S[:, m, s:s + NB], ht[:])
                        nc.gpsimd.tensor_copy(S[:, 10 + m, s:s + NB], ht[:])
                    else:
                        cm = m - KH
                        nc.vector.tensor_scalar(C1[:, cm, s:s + NB], acc[:],
                                                BI[:, m:m + 1], None, OP.add)
                        nc.vector.tensor_copy(C2[:, cm, s:s + NB],
                                              C1[:, cm, s:s + NB])

            # ---- one LSTM cell for (layer, block, k-chunk) at parity par ----
            # DoubleRow matmuls accumulate i/f/o into a 3-bank PSUM tile and
            # g into its own bank; one wide sigmoid + one tanh produce fp16
            # gates in SBUF; the c update runs fp16 on DVE; h goes to the
            # fp16 staging tile and is cast to the fp8 state on GpSimd.
            def cell(layer, b, k, par):
                s = b * NB
                P3 = pp.tile([128, 3 * NB], F32, tag="P3")
                G1 = pp.tile([128, NB], F32, tag="G1")
                if layer == 0:
                    hb = 4 * par          # h1 from previous step
                    pairs = [(W0, 0, hb), (W0, 2, hb + 2), (W0, 4, 8)]
                else:
                    nb_ = 4 * (1 - par)   # h1 written THIS step by layer 0
                    hb = 10 + 4 * par     # h2 from previous step
                    pairs = [(W1, 0, nb_), (W1, 2, nb_ + 2),
                             (W1, 4, hb), (W1, 6, hb + 2),
                             (W1, 8, 18)]
                n = len(pairs)
                for slot in range(4):
                    col = slot * HID + k * 128
                    dst = P3[:, slot * NB:(slot + 1) * NB] if slot < 3 else G1[:]
                    for pi, (W, wt, st) in enumerate(pairs):
                        nc.tensor.matmul(dst, W[:, wt:wt + 2, col:col + 128],
                                         S[:, st:st + 2, s:s + NB],
                                         start=(pi == 0), stop=(pi == n - 1),
                                         perf_mode=DR)
                sifo = tp.tile([128, 3 * NB], F16, tag="sifo")
                gt = tp.tile([128, NB], F16, tag="gt")
                nc.scalar.activation(sifo[:], P3[:], AF.Sigmoid, scale=1.0 / SW)
                nc.scalar.activation(gt[:], G1[:], AF.Tanh, scale=1.0 / SW)
                cs = (C1 if layer == 0 else C2)[:, k, s:s + NB]
                ig = tp.tile([128, NB], F16, tag="ig")
                nc.vector.tensor_tensor(ig[:], sifo[:, 0:NB], gt[:], OP.mult)
                nc.vector.tensor_tensor(cs, sifo[:, NB:2 * NB], cs, OP.mult)
                nc.vector.tensor_tensor(cs, cs, ig[:], OP.add)
                tc_ = tp.tile([128, NB], F16, tag="tc")
                nc.scalar.activation(tc_[:], cs, AF.Tanh)
                hf = (H1F if layer == 0 else H2F)[:, k, s:s + NB]
                nc.vector.tensor_tensor(hf, sifo[:, 2 * NB:3 * NB], tc_[:],
                                        OP.mult)
                base = 4 * (1 - par) if layer == 0 else 10 + 4 * (1 - par)
                nc.gpsimd.tensor_copy(S[:, base + k, s:s + NB], hf)

            # ---- the autoregressive steps ----
            for t in range(unroll_T):
                par = t % 2
                for b in range(NBLK):
                    for k in range(KH):
                        cell(0, b, k, par)
                for b in range(NBLK):
                    for k in range(KH):
                        cell(1, b, k, par)
                    # fc_out for this block right away: x feeds the next
                    # step's layer-0 matmuls, so shorten its path
                    s = b * NB
                    acc = pp.tile([OUT, NB], F32, tag="G1")
                    for j in range(KH):
                        nc.tensor.matmul(acc[:], WO[:, j, :],
                                         H2F[:, j, s:s + NB],
                                         start=(j == 0), stop=(j == KH - 1))
                    nc.vector.tensor_scalar(XF[:, s:s + NB], acc[:], BO[:],
                                            None, OP.add)
                    nc.sync.dma_start(frames[t, :, s:s + NB], XF[:, s:s + NB])
                    nc.gpsimd.tensor_copy(S[0:OUT, 8, s:s + NB],
                                          XF[:, s:s + NB])

    nc.compile()
    return nc


def _q8(x):
    return np.asarray(x, dtype=np.float32).astype(E4M3)


def _prep_inputs(z, start_token, fc_init_w, fc_init_b,
                 w_ih0, w_hh0, b_ih0, b_hh0,
                 w_ih1, w_hh1, b_ih1, b_hh1,
                 fc_out_w, fc_out_b):
    f32 = np.float32
    H4 = 4 * HID
    perm = [0, 1, 3, 2]  # torch gate order i,f,g,o -> slot order i,f,o,g

    def reorder_rows(w):  # [4H, X] -> gate-slot-major rows
        return w.reshape(4, HID, -1)[perm].reshape(H4, -1)

    def hh_tiles(w, ntiles, off=0):
        # w [4H, K] -> fp8 tiles [128, ntiles, 4H] with tile j at off+j
        wr = reorder_rows(w)  # [4H, K]
        K = wr.shape[1]
        out = np.zeros((128, ntiles, H4), dtype=E4M3)
        for j in range(K // 128):
            out[:, off + j, :] = _q8(wr[:, j * 128:(j + 1) * 128].T * SW)
        return out

    def bias_rows(b):
        # fp8 bias on the ones row + 16x residual on the 1/16 row
        bs = reorder_rows(b[:, None])[:, 0] * SW
        hi = _q8(bs)
        lo = _q8(16.0 * (bs - hi.astype(np.float32)))
        return hi, lo

    # W0: tiles 0-3 = w_hh0; tile 4 rows 0-50 = w_ih0, rows 51/52 = bias
    w0 = np.zeros((128, 6, H4), dtype=E4M3)
    w0[:, 0:4, :] = hh_tiles(w_hh0, 4)[:, 0:4, :]
    w0[0:OUT, 4, :] = _q8(reorder_rows(w_ih0).T * SW)
    w0[OUT, 4, :], w0[OUT + 1, 4, :] = bias_rows(b_ih0 + b_hh0)
    # W1: tiles 0-3 = w_ih1 (vs h1), 4-7 = w_hh1 (vs h2), 8 = bias rows, 9 = 0
    w1 = np.zeros((128, 10, H4), dtype=E4M3)
    w1[:, 0:4, :] = hh_tiles(w_ih1, 4)[:, 0:4, :]
    w1[:, 4:8, :] = hh_tiles(w_hh1, 4)[:, 0:4, :]
    w1[0, 8, :], w1[1, 8, :] = bias_rows(b_ih1 + b_hh1)
    # WO fp16 [128, KH, OUT]
    wo = np.zeros((128, KH, OUT), dtype=np.float16)
    for j in range(KH):
        wo[:, j, :] = fc_out_w[:, j * 128:(j + 1) * 128].T.astype(np.float16)
    # s0: x tile 8 (start token rows 0-50, ones row 51), ones tile 18 row 0
    s0 = np.zeros((128, NS, BC), dtype=E4M3)
    s0[0:OUT, 8, :] = _q8(np.broadcast_to(start_token[:, None], (OUT, BC)))
    s0[OUT, 8, :] = E4M3(1.0)
    s0[OUT + 1, 8, :] = E4M3(0.0625)
    s0[0, 18, :] = E4M3(1.0)
    s0[1, 18, :] = E4M3(0.0625)

    common = {
        "w0d": w0,
        "w1d": w1,
        "wod": wo,
        "wid": np.ascontiguousarray(fc_init_w.T, dtype=f32),
        "bid": np.ascontiguousarray(fc_init_b.reshape(8, 128).T, dtype=f32),
        "bod": np.ascontiguousarray(fc_out_b[:, None], dtype=f32),
        "s0": s0,
    }
    in_maps = []
    for c in range(NCORES):
        m = dict(common)
        m["zT"] = np.ascontiguousarray(z[c * BC:(c + 1) * BC].T, dtype=f32)
        in_maps.append(m)
    return in_maps


def kernel(**inputs):
    if "nc" not in _cached:
        _cached["nc"] = build_module()
    nc = _cached["nc"]
    in_maps = _prep_inputs(**inputs)
    res = run_bass_kernel_spmd(nc, in_maps, list(range(NCORES)))
    # frames per core: [T, OUT, BC] -> full [B, T, OUT]
    out = np.stack([res.results[c]["frames"] for c in range(NCORES)])
    return np.ascontiguousarray(
        out.transpose(0, 3, 1, 2).reshape(B, T, OUT))
